# revision 38
# baseline (speedup 1.0000x reference)
"""Trainium2 Bass kernel for margin-ranking + weighted-BCE loss pair.

Math
----
reference margin loss (labels are 0/1):
  S_full := sum_{i,j in [B]^2} relu(m - prod_ij),  prod_ij = (p_i-p_j)(l_i-l_j)
  margin_loss = S_full/(2B) - relu(m)/2
prod is symmetric and zero for same-label pairs, so with d = p_pos - p_neg:
  S_full = m*(Npos^2 + Nneg^2) + 2 * sum_{i in pos, j in neg} relu(m - d_ij)

Device computation: with posm := pos - m (bf16-rounded on the host),
  relu(m - d) = relu(neg - posm) = max(neg, posm) - posm
so a 128-pos x 1024-neg tile of the cross grid is one fused max() op, and
the -posm shift is removed exactly on the host via the device-reduced posm
sum.  max(bf16, bf16) is exact, so the only rounding is f32 summation.

The 17 tiles per core are split across the three engines that can do this
work at speed (HW-measured rates; the DVE accumulate path runs at 1x, so
production at ~0.49us/tile (4x mode) is separated from reduction):
  - 12 tiles: DVE plain tensor_scalar produces the max-tile, PE ones-vector
    matmuls accumulate column sums into a [1,128] PSUM row (~0.10us per
    128-col matmul warm; dummy matmuls during the input-DMA window start
    the PE clock ramp early)
  - 4 tiles: fused Act activation relu(negrep - posm) via bias=-posm with
    accum_out (~1.41us; exact relu, no posm correction)
  - 1 tile: fused DVE tensor_scalar + accum (1 instr, ~1.27us)
(Pool's AP-scalar tensor_scalar measured ~15us/tile - unusable.)

The result leaves as raw per-partition partials ([128,10] acc + [1,128]
psum row) straight from the accumulators - no on-device partition
reduction - and the host does the final sums in f64 (as the baseline did).

BCE: bce_i = (1-t)z + (1+(pw-1)t)*softplus(-z), softplus(-z) = ln(1+exp(-z))
(safe: |z| tiny).  Exp shares the initial act table; the Ln table swap is
placed after the grid relu blocks so it never stalls them.

Distribution: core c = (q, h) owns pos half q (17 blocks of 128) x neg
quarter h (1024 cols).  Host does only permutation/padding/replication and
the final combine of per-core partial sums.
"""

import numpy as np
import ml_dtypes

import concourse.bacc as bacc
import concourse.bass as bass
import concourse.mybir as mybir
import concourse.tile as tile
from concourse.bass_utils import run_bass_kernel_spmd

B = 8192
NCORES = 8
PP = 4352                  # padded pos count: 34 blocks of 128, 17 per core
NN = 4096                  # padded neg count: 4 quarters of 1024
NB = 17                    # pos blocks per core
W = 1024                   # neg cols per core
SP = 16.0                  # pos sentinel
SN = -16.0                 # neg sentinel
BCE_N = B // NCORES        # 1024 -> [128, 8]
BCE_F = BCE_N // 128
NWARM = 6

# (role, posm col) per block in emission order.  P: DVE tensor_scalar
# produces the max-tile, PE ones-matmuls sum it into the psum row.
# F: fused DVE tensor_scalar with accum_out.  A: fused Act activation
# relu(negrep - posm) with accum_out (exact relu; no posm correction).
# P/F blocks use posm cols [0:13) so the correction reduce is contiguous.
ROLES = [("P", 0), ("A", 13), ("P", 1), ("P", 2), ("A", 14), ("P", 3),
         ("P", 4), ("P", 12), ("P", 5), ("A", 15), ("P", 6), ("P", 7),
         ("P", 8), ("A", 16), ("P", 9), ("P", 10), ("P", 11)]
assert len(ROLES) == NB
assert sorted(b for _, b in ROLES) == list(range(NB))
N_PE = sum(1 for r, _ in ROLES if r == "P")
N_ACT = sum(1 for r, _ in ROLES if r == "A")
N_F = sum(1 for r, _ in ROLES if r == "F")
NCORR = N_PE + N_F         # blocks needing the W*posm correction (cols 0..)
NMARG = N_ACT + N_F        # margin accum cols (PE blocks go to the psum row)
# acc col layout: [0:NMARG) margin, then sp, tsp, tz, z, posm_corr, pe_total
# (pe_total lives on partition 0 only)
NACC = NMARG + 6

f32 = mybir.dt.float32
bf16 = mybir.dt.bfloat16


def _build_program(margin: float):
    from contextlib import ExitStack

    assert 0.0 <= margin <= 8.0, "sentinel padding assumes 0 <= margin <= 8"
    nc = bacc.Bacc("TRN2", target_bir_lowering=False, debug=False,
                   num_devices=NCORES)
    Relu = mybir.ActivationFunctionType.Relu
    Exp = mybir.ActivationFunctionType.Exp
    Ln = mybir.ActivationFunctionType.Ln
    add = mybir.AluOpType.add
    mult = mybir.AluOpType.mult
    amax = mybir.AluOpType.max
    bypass = mybir.AluOpType.bypass

    # aux packs posm | logits | targets into one DMA
    negrep_d = nc.dram_tensor("negrep", [128, W], bf16, kind="ExternalInput")
    aux_d = nc.dram_tensor("aux", [128, NB + 2 * BCE_F], f32,
                           kind="ExternalInput")
    out_d = nc.dram_tensor("out", [128, NACC], f32, kind="ExternalOutput")

    with tile.TileContext(nc) as tc, ExitStack() as ctx:
        small = ctx.enter_context(tc.tile_pool(name="small", bufs=1))
        dpool = ctx.enter_context(tc.tile_pool(name="dpool", bufs=6))
        spool = ctx.enter_context(tc.tile_pool(name="spool", bufs=2))
        psum = ctx.enter_context(
            tc.tile_pool(name="psum", bufs=2, space=bass.MemorySpace.PSUM))
        psmall = ctx.enter_context(
            tc.tile_pool(name="psmall", bufs=1, space=bass.MemorySpace.PSUM))

        # ---- prologue: warmup constants first, then input DMAs -----------
        # wtile memset leads the Pool stream so the PE warmup matmuls (which
        # coax the PE clock up from its idle pstate) start immediately
        wtile = small.tile([128, 512], bf16, tag="wtile")
        onesb = small.tile([128, 1], bf16, tag="onesb")
        nc.gpsimd.memset(wtile[:, :], 0.0)
        nc.gpsimd.memset(onesb[:, :], 1.0)
        for _ in range(NWARM):
            wpsum = psum.tile([128, 512], f32, tag="warm")
            nc.tensor.matmul(wpsum[:, :], wtile[:, 0:128], wtile[:, :],
                             start=True, stop=True)

        negrep = small.tile([128, W], bf16, tag="negrep")
        aux = small.tile([128, NB + 2 * BCE_F], f32, tag="aux")
        nc.sync.dma_start(out=negrep[:, :], in_=negrep_d[:, :])
        nc.scalar.dma_start(out=aux[:, :], in_=aux_d[:, :])
        posm = aux[:, 0:NB]
        zt = aux[:, NB:NB + BCE_F]
        tt = aux[:, NB + BCE_F:NB + 2 * BCE_F]

        # negposm = -posm, the per-partition relu bias for the Act blocks
        # (on Pool so DVE can start grid tiles immediately)
        negposm = small.tile([128, NB], f32, tag="negposm")
        nc.gpsimd.tensor_scalar_mul(negposm[:, :], posm, -1.0)

        acc = small.tile([128, NACC], f32, tag="acc")
        # pe_total column is only written on partition 0; zero the rest
        nc.gpsimd.memset(acc[:, NMARG + 5: NMARG + 6], 0.0)

        # ---- BCE, part 1: exp(-z) on Act (same act table as Relu) --------
        sp = small.tile([128, BCE_F], f32, tag="sp")
        e1 = small.tile([128, BCE_F], f32, tag="e1")
        scr8a = small.tile([128, BCE_F], f32, tag="scr8a")
        scr8b = small.tile([128, BCE_F], f32, tag="scr8b")
        nc.scalar.activation(e1[:, :], zt, Exp, scale=-1.0)
        nc.vector.scalar_tensor_tensor(
            scr8b[:, :], tt, 1.0, zt, bypass, mult,
            accum_out=acc[:, NMARG + 2: NMARG + 3])
        nc.vector.tensor_reduce(acc[:, NMARG + 3: NMARG + 4], zt,
                                axis=mybir.AxisListType.X, op=add)
        # per-partition posm correction sum for the P/F blocks
        nc.vector.tensor_reduce(acc[:, NMARG + 4: NMARG + 5],
                                posm[:, 0:NCORR],
                                axis=mybir.AxisListType.X, op=add)

        # ---- the 17 pos-block x 1024-neg-col grid tiles ------------------
        # tile[p, n] = max(negrep[n], posm[p, b]); sum(tile) - W*posm is the
        # block's relu sum (host removes the shift via the posm correction)
        pesum = psmall.tile([1, 512], f32, tag="pesum")
        n_pe_mm = N_PE * (W // 512)
        im = 0
        imarg = 0
        n_act = 0
        for role, b in ROLES:
            pv = posm[:, b: b + 1]
            if role == "F":
                scr = spool.tile([128, W], bf16, tag="scr")
                nc.vector.tensor_scalar(scr[:, :], negrep[:, :], pv,
                                        0.0, amax, add,
                                        accum_out=acc[:, imarg: imarg + 1])
                imarg += 1
            elif role == "A":
                ascr = spool.tile([128, W], bf16, tag="ascr")
                nc.scalar.activation(ascr[:, :], negrep[:, :], Relu,
                                     bias=negposm[:, b: b + 1],
                                     accum_out=acc[:, imarg: imarg + 1])
                imarg += 1
                n_act += 1
                if n_act == 1:
                    # BCE part 2 rides between relu blocks: Relu survives
                    # the Ln act-table swap, so the swap cost sits here in
                    # the middle of the Act stream instead of in the tail
                    nc.scalar.activation(sp[:, :], e1[:, :], Ln, bias=1.0,
                                         accum_out=acc[:, NMARG: NMARG + 1])
                    nc.vector.scalar_tensor_tensor(
                        scr8a[:, :], tt, 1.0, sp[:, :], bypass, mult,
                        accum_out=acc[:, NMARG + 1: NMARG + 2])
            else:  # P: DVE produces, PE ones-matmuls consume
                dt = dpool.tile([128, W], bf16, tag="dtile")
                nc.vector.tensor_scalar(dt[:, :], negrep[:, :], pv,
                                        0.0, amax, add)
                for j in range(W // 512):
                    nc.tensor.matmul(pesum[:, :], onesb[:, :],
                                     dt[:, 512 * j: 512 * (j + 1)],
                                     start=(im == 0), stop=(im == n_pe_mm - 1))
                    im += 1
        assert imarg == NMARG and im == n_pe_mm

        # ---- ship raw partials; host does the final reduction -----------
        # the PE psum row collapses to one cell on partition 0 of acc
        nc.vector.tensor_reduce(acc[0:1, NMARG + 5: NMARG + 6], pesum[:, :],
                                axis=mybir.AxisListType.X, op=add)
        nc.sync.dma_start(out=out_d[:, :], in_=acc[:, :])

    nc.compile()
    return nc


_programs: dict = {}


def _get_program(margin: float):
    if margin not in _programs:
        _programs[margin] = _build_program(margin)
    return _programs[margin]


def _make_in_maps(preds, labels, logits, targets, margin):
    p = np.ascontiguousarray(np.asarray(preds, np.float32))
    l = np.ascontiguousarray(np.asarray(labels, np.float32))
    z = np.ascontiguousarray(np.asarray(logits, np.float32))
    tg = np.ascontiguousarray(np.asarray(targets, np.float32))
    ndt = ml_dtypes.bfloat16

    mask = l >= 0.5
    npos = int(mask.sum())
    nneg = B - npos
    assert npos <= PP and nneg <= NN, (npos, nneg)
    pos_pad = np.full(PP, SP, np.float32)
    pos_pad[:npos] = p[mask]
    neg_pad = np.full(NN, SN, np.float32)
    neg_pad[:nneg] = p[~mask]
    neg16 = neg_pad.astype(ndt)
    # posm rounded to bf16 so that max(neg16, posm) on device is exact
    posm = (pos_pad - np.float32(margin)).astype(ndt).astype(np.float32)

    in_maps = []
    for c in range(NCORES):
        q, h = divmod(c, 4)
        aux = np.empty((128, NB + 2 * BCE_F), np.float32)
        aux[:, 0:NB] = posm[q * NB * 128:(q + 1) * NB * 128].reshape(NB, 128).T
        aux[:, NB:NB + BCE_F] = \
            z[BCE_N * c: BCE_N * (c + 1)].reshape(128, BCE_F)
        aux[:, NB + BCE_F:NB + 2 * BCE_F] = \
            tg[BCE_N * c: BCE_N * (c + 1)].reshape(128, BCE_F)
        negs = neg16[h * W:(h + 1) * W]
        in_maps.append({
            "negrep": np.ascontiguousarray(np.broadcast_to(negs, (128, W))),
            "aux": aux,
        })
    return in_maps, npos, nneg


def _combine(accs, npos, nneg, margin, pw):
    # accs: [NCORES, 128, NACC] raw per-partition partials.  acc cols:
    # [0:NMARG) margin (A relu sums), then sp, t*sp, t*z, z, posm_corr,
    # pe_total (partition 0 only).
    a = accs.astype(np.float64)
    m = float(margin)
    s_grid = (a[:, :, 0:NMARG].sum() + a[:, 0, NMARG + 5].sum()
              - W * a[:, :, NMARG + 4].sum())
    s_full = m * (npos * npos + nneg * nneg) + 2.0 * s_grid
    margin_loss = s_full / (2.0 * B) - max(m, 0.0) / 2.0
    s_bce = (a[:, :, NMARG + 3].sum() - a[:, :, NMARG + 2].sum()
             + a[:, :, NMARG + 0].sum() + (pw - 1.0) * a[:, :, NMARG + 1].sum())
    return np.array([margin_loss, s_bce / B], dtype=np.float32)


def _run(inputs: dict, trace: bool = False, **spmd_kwargs):
    m = float(np.asarray(inputs["margin"]))
    pw = float(np.asarray(inputs["pos_weight"], np.float32).reshape(-1)[0])
    nc = _get_program(m)
    in_maps, npos, nneg = _make_in_maps(inputs["preds"], inputs["labels"],
                                        inputs["logits"], inputs["targets"],
                                        m)
    res = run_bass_kernel_spmd(nc, in_maps, core_ids=list(range(NCORES)),
                               trace=trace, **spmd_kwargs)
    accs = np.stack([np.asarray(r["out"], np.float32) for r in res.results])
    return _combine(accs, npos, nneg, m, pw), res


def kernel(preds, labels, logits, targets, pos_weight, margin):
    out, _ = _run(dict(preds=preds, labels=labels, logits=logits,
                       targets=targets, pos_weight=pos_weight,
                       margin=margin))
    return out


# revision 51
# speedup vs baseline: 1.0269x; 1.0269x over previous
"""Trainium2 Bass kernel for margin-ranking + weighted-BCE loss pair.

Math
----
reference margin loss (labels are 0/1):
  S_full := sum_{i,j in [B]^2} relu(m - prod_ij),  prod_ij = (p_i-p_j)(l_i-l_j)
  margin_loss = S_full/(2B) - relu(m)/2
prod is symmetric and zero for same-label pairs, so with d = p_pos - p_neg:
  S_full = m*(Npos^2 + Nneg^2) + 2 * sum_{i in pos, j in neg} relu(m - d_ij)

Device computation: with posm := pos - m (bf16-rounded on the host),
  relu(m - d) = relu(neg - posm) = max(neg, posm) - posm
so a 128-pos x 1024-neg tile of the cross grid is one fused max() op, and
the -posm shift is removed exactly on the host via the device-reduced posm
sum.  max(bf16, bf16) is exact, so the only rounding is f32 summation.

The 17 tiles per core are split across the engines that can do this work
at speed (HW-measured rates; the DVE accumulate path runs at 1x, so
production at ~0.49us/tile (4x DVE mode) is separated from reduction):
  - 12 tiles: DVE plain tensor_scalar produces the max-tile, PE ones-vector
    matmuls accumulate column sums into a [1,128] PSUM row.  The PE's HAM
    duty-cycle limit (4/8 until a utilization grant arrives mid-run) is the
    main run-to-run variance; dummy matmuls during the input-DMA window
    pull the grant earlier.
  - 5 tiles: fused Act activation relu(negrep - posm) via bias=-posm with
    accum_out (~1.41us; exact relu, no posm correction)
(Pool's AP-scalar tensor_scalar measured ~15us/tile - unusable; DVE fused
tensor_scalar+accum at 1.27us/tile is kept as the "F" role but unused.)

The result leaves as raw per-partition partials ([128,10] acc + [1,128]
psum row) straight from the accumulators - no on-device partition
reduction - and the host does the final sums in f64 (as the baseline did).

BCE: bce_i = (1-t)z + (1+(pw-1)t)*softplus(-z), softplus(-z) = ln(1+exp(-z))
(safe: |z| tiny).  Exp shares the initial act table; the Ln table swap is
placed after the grid relu blocks so it never stalls them.

Distribution: core c = (q, h) owns pos half q (17 blocks of 128) x neg
quarter h (1024 cols).  Host does only permutation/padding/replication and
the final combine of per-core partial sums.
"""

import numpy as np
import ml_dtypes

import concourse.bacc as bacc
import concourse.bass as bass
import concourse.mybir as mybir
import concourse.tile as tile
from concourse.bass_utils import run_bass_kernel_spmd

B = 8192
NCORES = 8
PP = 4352                  # padded pos count: 34 blocks of 128, 17 per core
NN = 4096                  # padded neg count: 4 quarters of 1024
NB = 17                    # pos blocks per core
W = 1024                   # neg cols per core
SP = 16.0                  # pos sentinel
SN = -16.0                 # neg sentinel
BCE_N = B // NCORES        # 1024 -> [128, 8]
BCE_F = BCE_N // 128
NWARM = 7

# (role, posm col) per block in emission order.  P: DVE tensor_scalar
# produces the max-tile, PE ones-matmuls sum it into the psum row.
# F: fused DVE tensor_scalar with accum_out.  A: fused Act activation
# relu(negrep - posm) with accum_out (exact relu; no posm correction).
# P/F blocks use posm cols [0:13) so the correction reduce is contiguous.
ROLES = [("P", 0), ("A", 12), ("P", 1), ("P", 2), ("A", 13), ("P", 3),
         ("P", 4), ("A", 14), ("P", 5), ("A", 15), ("P", 6), ("P", 7),
         ("P", 8), ("A", 16), ("P", 9), ("P", 10), ("P", 11)]
assert len(ROLES) == NB
assert sorted(b for _, b in ROLES) == list(range(NB))
N_PE = sum(1 for r, _ in ROLES if r == "P")
N_ACT = sum(1 for r, _ in ROLES if r == "A")
N_F = sum(1 for r, _ in ROLES if r == "F")
NCORR = N_PE + N_F         # blocks needing the W*posm correction (cols 0..)
NMARG = N_ACT + N_F        # margin accum cols (PE blocks go to the psum row)
# acc col layout: [0:NMARG) margin, then sp, tsp, tz, z, posm_corr, pe_total
# (pe_total lives on partition 0 only)
NACC = NMARG + 6

f32 = mybir.dt.float32
bf16 = mybir.dt.bfloat16


def _build_program(margin: float):
    from contextlib import ExitStack

    assert 0.0 <= margin <= 8.0, "sentinel padding assumes 0 <= margin <= 8"
    nc = bacc.Bacc("TRN2", target_bir_lowering=False, debug=False,
                   num_devices=NCORES)
    Relu = mybir.ActivationFunctionType.Relu
    Exp = mybir.ActivationFunctionType.Exp
    Ln = mybir.ActivationFunctionType.Ln
    add = mybir.AluOpType.add
    mult = mybir.AluOpType.mult
    amax = mybir.AluOpType.max
    bypass = mybir.AluOpType.bypass

    # aux packs posm | logits | targets into one DMA
    negrep_d = nc.dram_tensor("negrep", [128, W], bf16, kind="ExternalInput")
    aux_d = nc.dram_tensor("aux", [128, NB + 2 * BCE_F], f32,
                           kind="ExternalInput")
    out_d = nc.dram_tensor("out", [128, NACC], f32, kind="ExternalOutput")

    with tile.TileContext(nc) as tc, ExitStack() as ctx:
        small = ctx.enter_context(tc.tile_pool(name="small", bufs=1))
        dpool = ctx.enter_context(tc.tile_pool(name="dpool", bufs=10))
        spool = ctx.enter_context(tc.tile_pool(name="spool", bufs=2))
        psum = ctx.enter_context(
            tc.tile_pool(name="psum", bufs=2, space=bass.MemorySpace.PSUM))
        psmall = ctx.enter_context(
            tc.tile_pool(name="psmall", bufs=1, space=bass.MemorySpace.PSUM))

        # ---- prologue: warmup constants first, then input DMAs -----------
        # wtile memset leads the Pool stream so the PE warmup matmuls (which
        # coax the PE clock up from its idle pstate) start immediately
        wtile = small.tile([128, 512], bf16, tag="wtile")
        onesb = small.tile([128, 1], bf16, tag="onesb")
        nc.scalar.memzero(wtile[:, :])
        nc.gpsimd.memset(onesb[:, :], 1.0)
        for _ in range(NWARM):
            wpsum = psum.tile([128, 512], f32, tag="warm")
            nc.tensor.matmul(wpsum[:, :], wtile[:, 0:128], wtile[:, :],
                             start=True, stop=True)

        negrep = small.tile([128, W], bf16, tag="negrep")
        aux = small.tile([128, NB + 2 * BCE_F], f32, tag="aux")
        nc.sync.dma_start(out=negrep[:, :], in_=negrep_d[:, :])
        nc.scalar.dma_start(out=aux[:, :], in_=aux_d[:, :])
        posm = aux[:, 0:NB]
        zt = aux[:, NB:NB + BCE_F]
        tt = aux[:, NB + BCE_F:NB + 2 * BCE_F]

        # negposm = -posm, the per-partition relu bias for the Act blocks
        # (on Pool so DVE can start grid tiles immediately)
        negposm = small.tile([128, NB], f32, tag="negposm")
        nc.gpsimd.tensor_scalar_mul(negposm[:, :], posm, -1.0)

        acc = small.tile([128, NACC], f32, tag="acc")
        # pe_total column is only written on partition 0; zero the rest
        nc.gpsimd.memset(acc[:, NMARG + 5: NMARG + 6], 0.0)

        # ---- BCE, part 1: exp(-z) on Act (same act table as Relu) --------
        sp = small.tile([128, BCE_F], f32, tag="sp")
        e1 = small.tile([128, BCE_F], f32, tag="e1")
        scr8a = small.tile([128, BCE_F], f32, tag="scr8a")
        scr8b = small.tile([128, BCE_F], f32, tag="scr8b")
        nc.scalar.activation(e1[:, :], zt, Exp, scale=-1.0)

        # ---- the 17 pos-block x 1024-neg-col grid tiles ------------------
        # tile[p, n] = max(negrep[n], posm[p, b]); sum(tile) - W*posm is the
        # block's relu sum (host removes the shift via the posm correction)
        pesum = psmall.tile([1, 128], f32, tag="pesum")
        n_pe_mm = N_PE * (W // 128)
        im = 0
        imarg = 0
        n_act = 0
        for role, b in ROLES:
            pv = posm[:, b: b + 1]
            if role == "F":
                scr = spool.tile([128, W], bf16, tag="scr")
                nc.vector.tensor_scalar(scr[:, :], negrep[:, :], pv,
                                        0.0, amax, add,
                                        accum_out=acc[:, imarg: imarg + 1])
                imarg += 1
            elif role == "A":
                ascr = spool.tile([128, W], bf16, tag="ascr")
                nc.scalar.activation(ascr[:, :], negrep[:, :], Relu,
                                     bias=negposm[:, b: b + 1],
                                     accum_out=acc[:, imarg: imarg + 1])
                imarg += 1
                n_act += 1
                if n_act == 1:
                    # BCE part 2 rides between relu blocks: Relu survives
                    # the Ln act-table swap, so the swap cost sits here in
                    # the middle of the Act stream instead of in the tail
                    nc.scalar.activation(sp[:, :], e1[:, :], Ln, bias=1.0,
                                         accum_out=acc[:, NMARG: NMARG + 1])
                    nc.vector.scalar_tensor_tensor(
                        scr8a[:, :], tt, 1.0, sp[:, :], bypass, mult,
                        accum_out=acc[:, NMARG + 1: NMARG + 2])
            else:  # P: DVE produces, PE ones-matmuls consume
                dt = dpool.tile([128, W], bf16, tag="dtile")
                nc.vector.tensor_scalar(dt[:, :], negrep[:, :], pv,
                                        0.0, amax, add)
                for j in range(W // 128):
                    nc.tensor.matmul(pesum[:, :], onesb[:, :],
                                     dt[:, 128 * j: 128 * (j + 1)],
                                     start=(im == 0), stop=(im == n_pe_mm - 1))
                    im += 1
        assert imarg == NMARG and im == n_pe_mm

        # remaining small DVE reductions, after the grid tiles so they
        # don't delay the first tile
        nc.vector.scalar_tensor_tensor(
            scr8b[:, :], tt, 1.0, zt, bypass, mult,
            accum_out=acc[:, NMARG + 2: NMARG + 3])
        nc.vector.tensor_reduce(acc[:, NMARG + 3: NMARG + 4], zt,
                                axis=mybir.AxisListType.X, op=add)
        # per-partition posm correction sum for the P/F blocks
        nc.vector.tensor_reduce(acc[:, NMARG + 4: NMARG + 5],
                                posm[:, 0:NCORR],
                                axis=mybir.AxisListType.X, op=add)

        # ---- ship raw partials; host does the final reduction -----------
        # the PE psum row collapses to one cell on partition 0 of acc
        nc.vector.tensor_reduce(acc[0:1, NMARG + 5: NMARG + 6], pesum[:, :],
                                axis=mybir.AxisListType.X, op=add)
        nc.sync.dma_start(out=out_d[:, :], in_=acc[:, :])

    nc.compile()
    return nc


_programs: dict = {}


def _get_program(margin: float):
    if margin not in _programs:
        _programs[margin] = _build_program(margin)
    return _programs[margin]


def _make_in_maps(preds, labels, logits, targets, margin):
    p = np.ascontiguousarray(np.asarray(preds, np.float32))
    l = np.ascontiguousarray(np.asarray(labels, np.float32))
    z = np.ascontiguousarray(np.asarray(logits, np.float32))
    tg = np.ascontiguousarray(np.asarray(targets, np.float32))
    ndt = ml_dtypes.bfloat16

    mask = l >= 0.5
    npos = int(mask.sum())
    nneg = B - npos
    assert npos <= PP and nneg <= NN, (npos, nneg)
    pos_pad = np.full(PP, SP, np.float32)
    pos_pad[:npos] = p[mask]
    neg_pad = np.full(NN, SN, np.float32)
    neg_pad[:nneg] = p[~mask]
    neg16 = neg_pad.astype(ndt)
    # posm rounded to bf16 so that max(neg16, posm) on device is exact
    posm = (pos_pad - np.float32(margin)).astype(ndt).astype(np.float32)

    in_maps = []
    for c in range(NCORES):
        q, h = divmod(c, 4)
        aux = np.empty((128, NB + 2 * BCE_F), np.float32)
        aux[:, 0:NB] = posm[q * NB * 128:(q + 1) * NB * 128].reshape(NB, 128).T
        aux[:, NB:NB + BCE_F] = \
            z[BCE_N * c: BCE_N * (c + 1)].reshape(128, BCE_F)
        aux[:, NB + BCE_F:NB + 2 * BCE_F] = \
            tg[BCE_N * c: BCE_N * (c + 1)].reshape(128, BCE_F)
        negs = neg16[h * W:(h + 1) * W]
        in_maps.append({
            "negrep": np.ascontiguousarray(np.broadcast_to(negs, (128, W))),
            "aux": aux,
        })
    return in_maps, npos, nneg


def _combine(accs, npos, nneg, margin, pw):
    # accs: [NCORES, 128, NACC] raw per-partition partials.  acc cols:
    # [0:NMARG) margin (A relu sums), then sp, t*sp, t*z, z, posm_corr,
    # pe_total (partition 0 only).
    a = accs.astype(np.float64)
    m = float(margin)
    s_grid = (a[:, :, 0:NMARG].sum() + a[:, 0, NMARG + 5].sum()
              - W * a[:, :, NMARG + 4].sum())
    s_full = m * (npos * npos + nneg * nneg) + 2.0 * s_grid
    margin_loss = s_full / (2.0 * B) - max(m, 0.0) / 2.0
    s_bce = (a[:, :, NMARG + 3].sum() - a[:, :, NMARG + 2].sum()
             + a[:, :, NMARG + 0].sum() + (pw - 1.0) * a[:, :, NMARG + 1].sum())
    return np.array([margin_loss, s_bce / B], dtype=np.float32)


def _run(inputs: dict, trace: bool = False, **spmd_kwargs):
    m = float(np.asarray(inputs["margin"]))
    pw = float(np.asarray(inputs["pos_weight"], np.float32).reshape(-1)[0])
    nc = _get_program(m)
    in_maps, npos, nneg = _make_in_maps(inputs["preds"], inputs["labels"],
                                        inputs["logits"], inputs["targets"],
                                        m)
    res = run_bass_kernel_spmd(nc, in_maps, core_ids=list(range(NCORES)),
                               trace=trace, **spmd_kwargs)
    accs = np.stack([np.asarray(r["out"], np.float32) for r in res.results])
    return _combine(accs, npos, nneg, m, pw), res


def kernel(preds, labels, logits, targets, pos_weight, margin):
    out, _ = _run(dict(preds=preds, labels=labels, logits=logits,
                       targets=targets, pos_weight=pos_weight,
                       margin=margin))
    return out
